# revision 31
# baseline (speedup 1.0000x reference)
"""Multi-head causal attention with RoPE on 8 Trainium2 NeuronCores.

Sharding: core c = 2*b + g handles batch b (of 4) and head-group g (of 2,
8 heads each).  Each core computes its 8 heads' attention and a partial
output projection (against its column-slice of wo); the host sums the two
partials per batch.

Per-core kernel layout notes:
 - x / wq / wk / wv / qt / kt are fp16 (PE rate identical to float32r,
   half the SBUF + DMA).  v / exp-scores / oT / wo stay float32r.
 - Phase A streams x ONCE per s-block and computes Q, K and V projections
   from the same resident x tile (all three weights stay in SBUF).
 - q/k head dims are permuted host-side (folded into wq/wk rows) so the
   RoPE rotate-half becomes a 16-row block swap that stream_shuffle can do
   in one DVE pass.  Scores are invariant to any fixed dim permutation.
 - kt is pair-stacked like qt (no zero padding); score matmuls contract
   K=64 per head (cycles = moving columns, so K=64 costs the same as a
   padded K=128 but saves the padding memsets and one RoPE add).
 - Scores are computed transposed (keys on partitions, queries free), so
   softmax's denominator comes free from an extra ones-column appended to V
   in the attn@V matmul, and exp() fuses with PSUM eviction on ScalarE.
 - Causal masking: key-tiles fully behind the query block are skipped, the
   triangular corner is zeroed with one precomputed [128,128] additive mask.
 - Softmax division is deferred: 1/denom = exp(-ln(denom)) on ScalarE
   (2 fast ACT table ops; DVE reciprocal is 3.2us), broadcast across
   partitions via a DRAM-bounce DMA, fused into the PSUM eviction multiply.
 - The output projection for a J-group is emitted one group late so its
   dependency on the normalize chain hides under attention matmuls; J runs
   ascending so the dense J=3 group forms the tail.
"""

import sys

sys.path.insert(0, "/opt/trn_rl_repo")

import numpy as np

D_MODEL = 1024
NUM_HEADS = 16
D_K = 64
B_FULL, S = 4, 2048
THETA = 10000.0
N_CORES = 8
H_CORE = 8  # heads per core
HP = 4      # head pairs per core
SB = 4      # 512-wide s-blocks
ST = 16     # 128-wide s-tiles
KT = 8      # 128-deep k-tiles over D_MODEL
MASK_NEG = -30000.0  # causal mask additive constant (fp16-representable)

# stream_shuffle applies its 32-entry mask within each 32-partition block:
# this swaps the two 16-row halves of every block.
SHUF16 = list(range(16, 32)) + list(range(0, 16))

_CACHE = {}


def _build_module(mm="float16"):
    import concourse.bacc as bacc
    import concourse.tile as tile
    from concourse import mybir
    from contextlib import ExitStack

    P = 128
    FP32 = mybir.dt.float32
    M16 = getattr(mybir.dt, mm)      # qk-side matmul dtype
    MR = mybir.dt.float32r          # v / attn / outproj matmul dtype
    EXP = mybir.ActivationFunctionType.Exp
    LN = mybir.ActivationFunctionType.Ln

    nc = bacc.Bacc("TRN2", target_bir_lowering=False, debug=False,
                   num_devices=N_CORES)

    xT = nc.dram_tensor("xT", [D_MODEL, S], M16, kind="ExternalInput")
    wqT = nc.dram_tensor("wqT", [D_MODEL, 512], M16, kind="ExternalInput")
    wkT = nc.dram_tensor("wkT", [D_MODEL, 512], M16, kind="ExternalInput")
    wvT = nc.dram_tensor("wvT", [D_MODEL, 512], M16, kind="ExternalInput")
    woT = nc.dram_tensor("woT", [512, D_MODEL], MR, kind="ExternalInput")
    cosT = nc.dram_tensor("cosT", [P, S], FP32, kind="ExternalInput")
    sinT = nc.dram_tensor("sinT", [P, S], FP32, kind="ExternalInput")
    maskA = nc.dram_tensor("maskA", [P, P], M16, kind="ExternalInput")
    identT = nc.dram_tensor("identT", [P, P], M16, kind="ExternalInput")
    outD = nc.dram_tensor("out", [S, D_MODEL], FP32, kind="ExternalOutput")
    # denominator bounce buffer: one row per (hp, J, head-of-pair)
    scr = nc.dram_tensor("scr", [HP * SB * 2, 512], FP32)

    xT3 = xT.rearrange("(ko p) s -> p ko s", p=P)
    wqT3 = wqT.rearrange("(ko p) m -> p ko m", p=P)
    wkT3 = wkT.rearrange("(ko p) m -> p ko m", p=P)
    wvT3 = wvT.rearrange("(ko p) m -> p ko m", p=P)
    woT3 = woT.rearrange("(t p) n -> p t n", p=P)

    with tile.TileContext(nc) as tc:
        with ExitStack() as ctx:
            const_pool = ctx.enter_context(tc.tile_pool(name="const", bufs=1))
            qk_pool = ctx.enter_context(tc.tile_pool(name="qk", bufs=1))
            v_pool = ctx.enter_context(tc.tile_pool(name="vp", bufs=1))

            maskadd_sb = const_pool.tile([P, P], M16, name="maskadd_sb")
            ident_sb = const_pool.tile([P, P], M16, name="ident_sb")
            nc.gpsimd.dma_start(out=maskadd_sb[:], in_=maskA[:, :])
            nc.gpsimd.dma_start(out=ident_sb[:], in_=identT[:, :])

            qt = [qk_pool.tile([P, S], M16, tag=f"qt{i}", name=f"qt{i}")
                  for i in range(HP)]
            kt = [qk_pool.tile([P, S], M16, tag=f"kt{i}", name=f"kt{i}")
                  for i in range(HP)]
            v_sb = v_pool.tile([P, ST, H_CORE, 65], MR)

            # ---------------- Phase A: QKV projections + RoPE ----------------
            # single x stream: all three weights resident, x loaded once per
            # 512-wide s-block, q/k/v computed from the same tile
            with ExitStack() as actx:
                wpool = actx.enter_context(tc.tile_pool(name="wts", bufs=1))
                xpool = actx.enter_context(tc.tile_pool(name="xs", bufs=2))
                cspool = actx.enter_context(tc.tile_pool(name="cs", bufs=1))
                rpool = actx.enter_context(tc.tile_pool(name="rope", bufs=2))
                psA = actx.enter_context(
                    tc.tile_pool(name="psA", bufs=3, space="PSUM"))

                cos_sb = cspool.tile([P, S], FP32, tag="cos", name="cos_sb")
                sin_sb = cspool.tile([P, S], FP32, tag="sin", name="sin_sb")
                # gpsimd queue: keep the sync queue free for w/x at startup
                nc.gpsimd.dma_start(out=cos_sb[:], in_=cosT[:, :])
                nc.gpsimd.dma_start(out=sin_sb[:], in_=sinT[:, :])
                # ones column (index 64) of every per-head V' block
                ones_c = cspool.tile([P, ST, H_CORE, 1], FP32, tag="ones",
                                     name="ones_c")
                nc.vector.memset(ones_c[:], 1.0)
                nc.scalar.copy(v_sb[:, :, :, 64:65], ones_c[:])

                # sync-queue order: wq, x(sb0), wk, wv -- the first q-matmul
                # only needs wq + x(sb0); wk/wv arrive during sb0's q-chain
                w_sb = {}
                for mode in ("q", "k", "v"):
                    w_sb[mode] = wpool.tile([P, KT, 512], M16, tag=f"w{mode}",
                                            name=f"w_{mode}")
                for k in range(KT):
                    nc.sync.dma_start(out=w_sb["q"][:, k, :],
                                      in_=wqT3[:, k, :])
                xs0 = xpool.tile([P, KT, 512], M16, tag="xs", name="xs")
                nc.sync.dma_start(out=xs0[:, 0:4, :], in_=xT3[:, 0:4, 0:512])
                nc.sync.dma_start(out=xs0[:, 4:8, :], in_=xT3[:, 4:8, 0:512])
                for wdram, mode in ((wkT3, "k"), (wvT3, "v")):
                    for k in range(KT):
                        nc.sync.dma_start(out=w_sb[mode][:, k, :],
                                          in_=wdram[:, k, :])

                for sb in range(SB):
                    sbs = slice(sb * 512, (sb + 1) * 512)
                    if sb == 0:
                        xs = xs0
                    else:
                        xs = xpool.tile([P, KT, 512], M16, tag="xs",
                                        name="xs")
                        nc.sync.dma_start(out=xs[:, 0:4, :],
                                          in_=xT3[:, 0:4, sbs])
                        nc.sync.dma_start(out=xs[:, 4:8, :],
                                          in_=xT3[:, 4:8, sbs])
                    for mode in ("q", "k"):
                        dst = qt if mode == "q" else kt
                        for hp in range(HP):
                            hps = slice(hp * 128, (hp + 1) * 128)
                            ps = psA.tile([P, 512], FP32, tag="pa",
                                          name="pa")
                            for k in range(KT):
                                nc.tensor.matmul(
                                    ps[:, :],
                                    w_sb[mode][:, k, hps],
                                    xs[:, k, :],
                                    start=(k == 0), stop=(k == KT - 1),
                                )
                            rot = rpool.tile([P, 512], FP32, tag="rot",
                                             name="rot")
                            nc.vector.stream_shuffle(rot[:], ps[:, :],
                                                     mask=SHUF16)
                            t1 = rpool.tile([P, 512], FP32, tag="t1",
                                            name="t1")
                            nc.vector.tensor_mul(t1[:], ps[:, :],
                                                 cos_sb[:, sbs])
                            t2 = rpool.tile([P, 512], FP32, tag="t2",
                                            name="t2")
                            nc.vector.tensor_mul(t2[:], rot[:],
                                                 sin_sb[:, sbs])
                            nc.vector.tensor_add(dst[hp][:, sbs],
                                                 t1[:], t2[:])
                    for sti in range(4):
                        st = sb * 4 + sti
                        stp = slice(sti * 128, (sti + 1) * 128)
                        psv = psA.tile([P, 512], FP32, tag="pa",
                                       name="psv")
                        for k in range(KT):
                            nc.tensor.matmul(
                                psv[:, :],
                                xs[:, k, stp],
                                w_sb["v"][:, k, :],
                                start=(k == 0), stop=(k == KT - 1),
                            )
                        pv = psv[:, :].rearrange("p (h d) -> p h d", h=8)
                        # every head: [V | 1]
                        nc.scalar.copy(v_sb[:, st, :, 0:64], pv[:, :, :])

            # ---------------- Phase B: attention ----------------
            ot_pool = ctx.enter_context(tc.tile_pool(name="otp", bufs=1))
            oT = [ot_pool.tile([P, S], MR, tag=f"oT{i}", name=f"oT{i}")
                  for i in range(HP)]
            wopool = ctx.enter_context(tc.tile_pool(name="wo", bufs=1))
            wo_sb = wopool.tile([P, 4, D_MODEL], MR)
            nc.sync.dma_start(out=wo_sb[:], in_=woT3[:, :, :])

            with ExitStack() as bctx:
                epool = bctx.enter_context(tc.tile_pool(name="expp", bufs=2))
                rdpool = bctx.enter_context(tc.tile_pool(name="rdp", bufs=2))
                bcpool = bctx.enter_context(tc.tile_pool(name="bcp", bufs=2))
                psS = bctx.enter_context(
                    tc.tile_pool(name="psS", bufs=2, space="PSUM"))
                psO = bctx.enter_context(
                    tc.tile_pool(name="psO", bufs=2, space="PSUM"))

                def nlo_of(I, J):
                    r = I - 4 * J
                    return 128 * r if r >= 0 else 0

                opool = bctx.enter_context(
                    tc.tile_pool(name="ostage", bufs=2))

                # output projection for one J-group's s-tiles; psum slots
                # borrowed from the scores pool (same tag).  Emitted one
                # J-group LATE so the softmax-normalize chain it depends on
                # hides under the next group's attention matmuls.
                def outproj(J):
                    for st in range(4 * J, 4 * J + 4):
                        stp = slice(st * 128, (st + 1) * 128)
                        pc = psS.tile([P, 2, 512], FP32, tag="psS",
                                      name="pc")
                        for nb in range(2):
                            nbs = slice(nb * 512, (nb + 1) * 512)
                            for t in range(4):
                                nc.tensor.matmul(
                                    pc[:, nb, :],
                                    oT[t][:, stp],
                                    wo_sb[:, t, nbs],
                                    start=(t == 0), stop=(t == 3),
                                )
                        ob = opool.tile([P, 2, 512], FP32, tag="ob",
                                        name="ob")
                        nc.scalar.copy(ob[:], pc[:, :])
                        nc.gpsimd.dma_start(
                            out=outD[stp, :],
                            in_=ob[:, :, :].rearrange("p a b -> p (a b)"))

                # J=1 first so J=0 (whose normalize chains outpace its tiny
                # PE workload) hides between J=1 and J=2; dense J=3 forms
                # the tail.  Round-robin over head pairs so eviction chains
                # hide under the next pair's matmuls.
                prev = []
                for J in (1, 0, 2, 3):
                    for hp in range(HP):
                        po = [psO.tile([P, 512], FP32, tag=f"po{h2}",
                                       name=f"po{h2}") for h2 in range(2)]
                        n_i = 4 * J + 4
                        for I in range(n_i):
                            nlo = nlo_of(I, J)
                            ks = slice(I * 128, (I + 1) * 128)
                            qs = slice(J * 512 + nlo, (J + 1) * 512)
                            diag = I - 4 * J >= 0
                            ps = psS.tile([P, 2, 512], FP32, tag="psS",
                                          name="psS")
                            for h2 in range(2):
                                hrows = slice(h2 * 64, (h2 + 1) * 64)
                                nc.tensor.matmul(
                                    ps[:, h2, nlo:],
                                    kt[hp][hrows, ks],
                                    qt[hp][hrows, qs],
                                    start=True, stop=not diag,
                                )
                                if diag:
                                    # causal corner: accumulate -30000 into
                                    # masked (q < k) entries, exp -> 0
                                    nc.tensor.matmul(
                                        ps[:, h2, nlo:nlo + 128],
                                        ident_sb[:, :],
                                        maskadd_sb[:, :],
                                        start=False, stop=True,
                                    )
                            ex = epool.tile([P, 2, 512], MR, tag="ex",
                                            name="ex")
                            for h2 in range(2):
                                # exp split per head: attn@V h2=0 starts
                                # after half the exp latency
                                nc.scalar.activation(ex[:, h2, nlo:],
                                                     ps[:, h2, nlo:],
                                                     EXP, scale=0.125)
                                # attn @ [V|1]: rows 0..63 dims, row 64 denom
                                nc.tensor.matmul(
                                    po[h2][0:65, nlo:],
                                    v_sb[:, I, hp * 2 + h2, :],
                                    ex[:, h2, nlo:],
                                    start=(I == 0), stop=(I == n_i - 1),
                                )
                        Js = slice(J * 512, (J + 1) * 512)
                        for h2 in range(2):
                            sidx = (hp * SB + J) * 2 + h2
                            # NOTE: exp(-ln(den)) on ScalarE would be faster
                            # per-op but thrashes the ACT table (Ln and Exp
                            # live in different default table sets; each
                            # switch costs a 1.3us ACT_TABLE_LOAD and
                            # serializes ScalarE against the scores exp)
                            # NOTE: reciprocal_approx_fast (custom DVE op)
                            # returns NaN in this runtime (uops table not
                            # applied) -- stick to the slow-but-sure op
                            rd = rdpool.tile([P, 512], FP32, tag="rd",
                                             name="rd")
                            nc.vector.reciprocal(rd[64:65, :],
                                                 po[h2][64:65, :])
                            nc.sync.dma_start(out=scr[sidx, :],
                                              in_=rd[64:65, :])
                            bc = bcpool.tile([P, 512], FP32, tag="bc",
                                             name="bc")
                            nc.sync.dma_start(
                                out=bc[0:64, :],
                                in_=scr[sidx, :].partition_broadcast(64))
                            if h2 == 0:
                                nc.vector.tensor_mul(
                                    oT[hp][0:64, Js],
                                    po[h2][0:64, :], bc[0:64, :])
                            else:
                                # normalized evict lands at partitions 0-63;
                                # DMA shifts it into oT's upper half
                                tmp = rdpool.tile([P, 512], MR, tag="tmpb",
                                                  name="tmpb")
                                nc.vector.tensor_mul(
                                    tmp[0:64, :], po[h2][0:64, :],
                                    bc[0:64, :])
                                nc.gpsimd.dma_start(out=oT[hp][64:128, Js],
                                                    in_=tmp[0:64, :])
                    if prev:
                        outproj(prev.pop())
                    prev.append(J)
                outproj(prev.pop())

    nc.compile()
    return nc


def get_module(mm="float16"):
    if mm not in _CACHE:
        _CACHE[mm] = _build_module(mm)
    return _CACHE[mm]


def _head_perm():
    """Within-head dim permutation: 16-pair blocks [x1 x2 x1 x2]."""
    p = []
    for blk in range(2):
        base = blk * 32
        p += [2 * (base // 2 + i) for i in range(16)]       # x1 of pairs
        p += [2 * (base // 2 + i) + 1 for i in range(16)]   # x2 of pairs
    return np.array(p)


def prep_core_inputs(inputs, mm="float16"):
    import ml_dtypes
    mdt = {"float16": np.float16, "bfloat16": ml_dtypes.bfloat16}.get(
        mm, np.float32)
    x = np.asarray(inputs["x"], dtype=np.float32)
    tp = np.asarray(inputs["token_positions"])
    wq = np.asarray(inputs["wq"], dtype=np.float32)
    wk = np.asarray(inputs["wk"], dtype=np.float32)
    wv = np.asarray(inputs["wv"], dtype=np.float32)
    wo = np.asarray(inputs["wo"], dtype=np.float32)

    perm = _head_perm()
    qi = np.arange(128)[None, :]
    ki = np.arange(128)[:, None]
    mask_add = np.where(qi < ki, np.float32(MASK_NEG),
                        np.float32(0.0)).astype(mdt)
    ident = np.eye(128, dtype=mdt)

    freqs = 1.0 / THETA ** (np.arange(0, D_K, 2, dtype=np.float32) / D_K)

    in_maps = []
    for c in range(N_CORES):
        b, g = divmod(c, 2)
        rows = slice(g * 512, (g + 1) * 512)
        wq_g = wq[rows].reshape(H_CORE, D_K, D_MODEL)[:, perm, :]
        wk_g = wk[rows].reshape(H_CORE, D_K, D_MODEL)[:, perm, :]

        pos = tp[b].astype(np.float32)
        ang = freqs[:, None] * pos[None, :]          # [32, S]
        cos32, sin32 = np.cos(ang), np.sin(ang)
        # permuted row l: l%32 < 16 -> x1 of pair (l%32 + 16*(l//32)),
        #                 else x2 of the same pair; x1 rows get -sin.
        cos64 = np.concatenate([cos32[0:16], cos32[0:16],
                                cos32[16:32], cos32[16:32]], axis=0)
        sin64 = np.concatenate([-sin32[0:16], sin32[0:16],
                                -sin32[16:32], sin32[16:32]], axis=0)
        cosT = np.tile(cos64, (2, 1))
        sinT = np.tile(sin64, (2, 1))

        in_maps.append({
            "xT": np.ascontiguousarray(x[b].T).astype(mdt),
            "wqT": np.ascontiguousarray(
                wq_g.reshape(512, D_MODEL).T).astype(mdt),
            "wkT": np.ascontiguousarray(
                wk_g.reshape(512, D_MODEL).T).astype(mdt),
            "wvT": np.ascontiguousarray(wv[rows].T).astype(mdt),
            "woT": np.ascontiguousarray(wo[:, rows].T).astype(np.float32),
            "cosT": np.ascontiguousarray(cosT),
            "sinT": np.ascontiguousarray(sinT),
            "maskA": mask_add,
            "identT": ident,
        })
    return in_maps


DEFAULT_MM = "float16"


def kernel(**inputs):
    from concourse.bass_utils import run_bass_kernel_spmd

    import os
    mm = os.environ.get("KMM", DEFAULT_MM)
    nc = get_module(mm)
    in_maps = prep_core_inputs(inputs, mm)
    res = run_bass_kernel_spmd(nc, in_maps, core_ids=list(range(N_CORES)))
    out = np.empty((B_FULL, S, D_MODEL), np.float32)
    for b in range(B_FULL):
        out[b] = res.results[2 * b]["out"] + res.results[2 * b + 1]["out"]
    return out


# revision 32
# speedup vs baseline: 1.2064x; 1.2064x over previous
"""Multi-head causal attention with RoPE on 8 Trainium2 NeuronCores.

Sharding: core c = 2*b + g handles batch b (of 4) and head-group g (of 2,
8 heads each).  Each core computes its 8 heads' attention and a partial
output projection (against its column-slice of wo); the host sums the two
partials per batch.

Per-core kernel layout notes:
 - x / wq / wk / wv / qt / kt are fp16 (PE rate identical to float32r,
   half the SBUF + DMA).  v / exp-scores / oT / wo stay float32r.
 - Phase A streams x ONCE per s-block and computes Q, K and V projections
   from the same resident x tile (all three weights stay in SBUF).
 - q/k head dims are permuted host-side (folded into wq/wk rows) so the
   RoPE rotate-half becomes a 16-row block swap that stream_shuffle can do
   in one DVE pass.  Scores are invariant to any fixed dim permutation.
 - kt is pair-stacked like qt (no zero padding); score matmuls contract
   K=64 per head (cycles = moving columns, so K=64 costs the same as a
   padded K=128 but saves the padding memsets and one RoPE add).
 - Scores are computed transposed (keys on partitions, queries free), so
   softmax's denominator comes free from an extra ones-column appended to V
   in the attn@V matmul, and exp() fuses with PSUM eviction on ScalarE.
 - Causal masking: key-tiles fully behind the query block are skipped, the
   triangular corner is zeroed with one precomputed [128,128] additive mask.
 - Softmax division is deferred: 1/denom = exp(-ln(denom)) on ScalarE
   (2 fast ACT table ops; DVE reciprocal is 3.2us), broadcast across
   partitions via a DRAM-bounce DMA, fused into the PSUM eviction multiply.
 - The output projection for a J-group is emitted one group late so its
   dependency on the normalize chain hides under attention matmuls; J runs
   ascending so the dense J=3 group forms the tail.
"""

import sys

sys.path.insert(0, "/opt/trn_rl_repo")

import numpy as np

D_MODEL = 1024
NUM_HEADS = 16
D_K = 64
B_FULL, S = 4, 2048
THETA = 10000.0
N_CORES = 8
H_CORE = 8  # heads per core
HP = 4      # head pairs per core
SB = 4      # 512-wide s-blocks
ST = 16     # 128-wide s-tiles
KT = 8      # 128-deep k-tiles over D_MODEL
MASK_NEG = -30000.0  # causal mask additive constant (fp16-representable)

# stream_shuffle applies its 32-entry mask within each 32-partition block:
# this swaps the two 16-row halves of every block.
SHUF16 = list(range(16, 32)) + list(range(0, 16))

_CACHE = {}


def _build_module(mm="float16"):
    import concourse.bacc as bacc
    import concourse.tile as tile
    from concourse import mybir
    from contextlib import ExitStack

    P = 128
    FP32 = mybir.dt.float32
    M16 = getattr(mybir.dt, mm)      # qk-side matmul dtype
    MR = mybir.dt.float32r          # v / attn / outproj matmul dtype
    EXP = mybir.ActivationFunctionType.Exp
    LN = mybir.ActivationFunctionType.Ln

    nc = bacc.Bacc("TRN2", target_bir_lowering=False, debug=False,
                   num_devices=N_CORES)

    xT = nc.dram_tensor("xT", [D_MODEL, S], M16, kind="ExternalInput")
    wqT = nc.dram_tensor("wqT", [D_MODEL, 512], M16, kind="ExternalInput")
    wkT = nc.dram_tensor("wkT", [D_MODEL, 512], M16, kind="ExternalInput")
    wvT = nc.dram_tensor("wvT", [D_MODEL, 512], M16, kind="ExternalInput")
    woT = nc.dram_tensor("woT", [512, D_MODEL], MR, kind="ExternalInput")
    cosT = nc.dram_tensor("cosT", [P, S], FP32, kind="ExternalInput")
    sinT = nc.dram_tensor("sinT", [P, S], FP32, kind="ExternalInput")
    maskA = nc.dram_tensor("maskA", [P, P], M16, kind="ExternalInput")
    identT = nc.dram_tensor("identT", [P, P], M16, kind="ExternalInput")
    outD = nc.dram_tensor("out", [S, D_MODEL], FP32, kind="ExternalOutput")
    # denominator bounce buffer: one row per (hp, J, head-of-pair)
    scr = nc.dram_tensor("scr", [HP * SB * 2, 512], FP32)

    xT3 = xT.rearrange("(ko p) s -> p ko s", p=P)
    wqT3 = wqT.rearrange("(ko p) m -> p ko m", p=P)
    wkT3 = wkT.rearrange("(ko p) m -> p ko m", p=P)
    wvT3 = wvT.rearrange("(ko p) m -> p ko m", p=P)
    woT3 = woT.rearrange("(t p) n -> p t n", p=P)

    with tile.TileContext(nc) as tc:
        with ExitStack() as ctx:
            const_pool = ctx.enter_context(tc.tile_pool(name="const", bufs=1))
            qk_pool = ctx.enter_context(tc.tile_pool(name="qk", bufs=1))
            v_pool = ctx.enter_context(tc.tile_pool(name="vp", bufs=1))

            maskadd_sb = const_pool.tile([P, P], M16, name="maskadd_sb")
            ident_sb = const_pool.tile([P, P], M16, name="ident_sb")
            nc.gpsimd.dma_start(out=maskadd_sb[:], in_=maskA[:, :])
            nc.gpsimd.dma_start(out=ident_sb[:], in_=identT[:, :])

            qt = [qk_pool.tile([P, S], M16, tag=f"qt{i}", name=f"qt{i}")
                  for i in range(HP)]
            kt = [qk_pool.tile([P, S], M16, tag=f"kt{i}", name=f"kt{i}")
                  for i in range(HP)]
            v_sb = v_pool.tile([P, ST, H_CORE, 65], MR)

            # ---------------- Phase A: QKV projections + RoPE ----------------
            # single x stream: all three weights resident, x loaded once per
            # 512-wide s-block, q/k/v computed from the same tile
            with ExitStack() as actx:
                wpool = actx.enter_context(tc.tile_pool(name="wts", bufs=1))
                xpool = actx.enter_context(tc.tile_pool(name="xs", bufs=2))
                cspool = actx.enter_context(tc.tile_pool(name="cs", bufs=1))
                rpool = actx.enter_context(tc.tile_pool(name="rope", bufs=2))
                psA = actx.enter_context(
                    tc.tile_pool(name="psA", bufs=3, space="PSUM"))

                cos_sb = cspool.tile([P, S], FP32, tag="cos", name="cos_sb")
                sin_sb = cspool.tile([P, S], FP32, tag="sin", name="sin_sb")
                # gpsimd queue: keep the sync queue free for w/x at startup
                nc.gpsimd.dma_start(out=cos_sb[:], in_=cosT[:, :])
                nc.gpsimd.dma_start(out=sin_sb[:], in_=sinT[:, :])
                # ones column (index 64) of every per-head V' block
                ones_c = cspool.tile([P, ST, H_CORE, 1], FP32, tag="ones",
                                     name="ones_c")
                nc.vector.memset(ones_c[:], 1.0)
                nc.scalar.copy(v_sb[:, :, :, 64:65], ones_c[:])

                # sync-queue order: wq, x(sb0), wk, wv -- the first q-matmul
                # only needs wq + x(sb0); wk/wv arrive during sb0's q-chain
                w_sb = {}
                for mode in ("q", "k", "v"):
                    w_sb[mode] = wpool.tile([P, KT, 512], M16, tag=f"w{mode}",
                                            name=f"w_{mode}")
                for k in range(KT):
                    nc.sync.dma_start(out=w_sb["q"][:, k, :],
                                      in_=wqT3[:, k, :])
                xs0 = xpool.tile([P, KT, 512], M16, tag="xs", name="xs")
                nc.sync.dma_start(out=xs0[:, 0:4, :], in_=xT3[:, 0:4, 0:512])
                nc.sync.dma_start(out=xs0[:, 4:8, :], in_=xT3[:, 4:8, 0:512])
                for wdram, mode in ((wkT3, "k"), (wvT3, "v")):
                    for k in range(KT):
                        nc.sync.dma_start(out=w_sb[mode][:, k, :],
                                          in_=wdram[:, k, :])

                for sb in range(SB):
                    sbs = slice(sb * 512, (sb + 1) * 512)
                    if sb == 0:
                        xs = xs0
                    else:
                        xs = xpool.tile([P, KT, 512], M16, tag="xs",
                                        name="xs")
                        nc.sync.dma_start(out=xs[:, 0:4, :],
                                          in_=xT3[:, 0:4, sbs])
                        nc.sync.dma_start(out=xs[:, 4:8, :],
                                          in_=xT3[:, 4:8, sbs])
                    for mode in ("q", "k"):
                        dst = qt if mode == "q" else kt
                        for hp in range(HP):
                            hps = slice(hp * 128, (hp + 1) * 128)
                            ps = psA.tile([P, 512], FP32, tag="pa",
                                          name="pa")
                            for k in range(KT):
                                nc.tensor.matmul(
                                    ps[:, :],
                                    w_sb[mode][:, k, hps],
                                    xs[:, k, :],
                                    start=(k == 0), stop=(k == KT - 1),
                                )
                            rot = rpool.tile([P, 512], FP32, tag="rot",
                                             name="rot")
                            nc.vector.stream_shuffle(rot[:], ps[:, :],
                                                     mask=SHUF16)
                            t1 = rpool.tile([P, 512], FP32, tag="t1",
                                            name="t1")
                            nc.vector.tensor_mul(t1[:], ps[:, :],
                                                 cos_sb[:, sbs])
                            t2 = rpool.tile([P, 512], FP32, tag="t2",
                                            name="t2")
                            nc.vector.tensor_mul(t2[:], rot[:],
                                                 sin_sb[:, sbs])
                            nc.vector.tensor_add(dst[hp][:, sbs],
                                                 t1[:], t2[:])
                    for sti in range(4):
                        st = sb * 4 + sti
                        stp = slice(sti * 128, (sti + 1) * 128)
                        psv = psA.tile([P, 512], FP32, tag="pa",
                                       name="psv")
                        for k in range(KT):
                            nc.tensor.matmul(
                                psv[:, :],
                                xs[:, k, stp],
                                w_sb["v"][:, k, :],
                                start=(k == 0), stop=(k == KT - 1),
                            )
                        pv = psv[:, :].rearrange("p (h d) -> p h d", h=8)
                        # every head: [V | 1]
                        nc.scalar.copy(v_sb[:, st, :, 0:64], pv[:, :, :])

            # ---------------- Phase B: attention ----------------
            ot_pool = ctx.enter_context(tc.tile_pool(name="otp", bufs=1))
            oT = [ot_pool.tile([P, S], MR, tag=f"oT{i}", name=f"oT{i}")
                  for i in range(HP)]
            wopool = ctx.enter_context(tc.tile_pool(name="wo", bufs=1))
            wo_sb = wopool.tile([P, 4, D_MODEL], MR)
            nc.sync.dma_start(out=wo_sb[:], in_=woT3[:, :, :])

            with ExitStack() as bctx:
                epool = bctx.enter_context(tc.tile_pool(name="expp", bufs=2))
                rdpool = bctx.enter_context(tc.tile_pool(name="rdp", bufs=2))
                bcpool = bctx.enter_context(tc.tile_pool(name="bcp", bufs=2))
                psS = bctx.enter_context(
                    tc.tile_pool(name="psS", bufs=2, space="PSUM"))
                psO = bctx.enter_context(
                    tc.tile_pool(name="psO", bufs=2, space="PSUM"))

                def nlo_of(I, J):
                    r = I - 4 * J
                    return 128 * r if r >= 0 else 0

                opool = bctx.enter_context(
                    tc.tile_pool(name="ostage", bufs=2))

                # output projection for one J-group's s-tiles; psum slots
                # borrowed from the scores pool (same tag).  Emitted one
                # J-group LATE so the softmax-normalize chain it depends on
                # hides under the next group's attention matmuls.
                def outproj(J):
                    for st in range(4 * J, 4 * J + 4):
                        stp = slice(st * 128, (st + 1) * 128)
                        pc = psS.tile([P, 2, 512], FP32, tag="psS",
                                      name="pc")
                        for nb in range(2):
                            nbs = slice(nb * 512, (nb + 1) * 512)
                            for t in range(4):
                                nc.tensor.matmul(
                                    pc[:, nb, :],
                                    oT[t][:, stp],
                                    wo_sb[:, t, nbs],
                                    start=(t == 0), stop=(t == 3),
                                )
                        ob = opool.tile([P, 2, 512], FP32, tag="ob",
                                        name="ob")
                        nc.scalar.copy(ob[:], pc[:, :])
                        nc.gpsimd.dma_start(
                            out=outD[stp, :],
                            in_=ob[:, :, :].rearrange("p a b -> p (a b)"))

                # J=1 first so J=0 (whose normalize chains outpace its tiny
                # PE workload) hides between J=1 and J=2; dense J=3 forms
                # the tail.  Round-robin over head pairs so eviction chains
                # hide under the next pair's matmuls.
                prev = []
                for J in (1, 0, 2, 3):
                    for hp in range(HP):
                        po = [psO.tile([P, 512], FP32, tag=f"po{h2}",
                                       name=f"po{h2}") for h2 in range(2)]
                        n_i = 4 * J + 4
                        for I in range(n_i):
                            nlo = nlo_of(I, J)
                            ks = slice(I * 128, (I + 1) * 128)
                            qs = slice(J * 512 + nlo, (J + 1) * 512)
                            diag = I - 4 * J >= 0
                            ps = psS.tile([P, 2, 512], FP32, tag="psS",
                                          name="psS")
                            for h2 in range(2):
                                hrows = slice(h2 * 64, (h2 + 1) * 64)
                                nc.tensor.matmul(
                                    ps[:, h2, nlo:],
                                    kt[hp][hrows, ks],
                                    qt[hp][hrows, qs],
                                    start=True, stop=not diag,
                                )
                                if diag:
                                    # causal corner: accumulate -30000 into
                                    # masked (q < k) entries, exp -> 0
                                    nc.tensor.matmul(
                                        ps[:, h2, nlo:nlo + 128],
                                        ident_sb[:, :],
                                        maskadd_sb[:, :],
                                        start=False, stop=True,
                                    )
                            ex = epool.tile([P, 2, 512], MR, tag="ex",
                                            name="ex")
                            nc.scalar.activation(ex[:, :, nlo:],
                                                 ps[:, :, nlo:],
                                                 EXP, scale=0.125)
                            for h2 in range(2):
                                # attn @ [V|1]: rows 0..63 dims, row 64 denom
                                nc.tensor.matmul(
                                    po[h2][0:65, nlo:],
                                    v_sb[:, I, hp * 2 + h2, :],
                                    ex[:, h2, nlo:],
                                    start=(I == 0), stop=(I == n_i - 1),
                                )
                        Js = slice(J * 512, (J + 1) * 512)
                        for h2 in range(2):
                            sidx = (hp * SB + J) * 2 + h2
                            # NOTE: exp(-ln(den)) on ScalarE would be faster
                            # per-op but thrashes the ACT table (Ln and Exp
                            # live in different default table sets; each
                            # switch costs a 1.3us ACT_TABLE_LOAD and
                            # serializes ScalarE against the scores exp)
                            # NOTE: reciprocal_approx_fast (custom DVE op)
                            # returns NaN in this runtime (uops table not
                            # applied) -- stick to the slow-but-sure op
                            rd = rdpool.tile([P, 512], FP32, tag="rd",
                                             name="rd")
                            nc.vector.reciprocal(rd[64:65, :],
                                                 po[h2][64:65, :])
                            nc.sync.dma_start(out=scr[sidx, :],
                                              in_=rd[64:65, :])
                            bc = bcpool.tile([P, 512], FP32, tag="bc",
                                             name="bc")
                            nc.sync.dma_start(
                                out=bc[0:64, :],
                                in_=scr[sidx, :].partition_broadcast(64))
                            if h2 == 0:
                                nc.vector.tensor_mul(
                                    oT[hp][0:64, Js],
                                    po[h2][0:64, :], bc[0:64, :])
                            else:
                                # normalized evict lands at partitions 0-63;
                                # DMA shifts it into oT's upper half
                                tmp = rdpool.tile([P, 512], MR, tag="tmpb",
                                                  name="tmpb")
                                nc.vector.tensor_mul(
                                    tmp[0:64, :], po[h2][0:64, :],
                                    bc[0:64, :])
                                nc.gpsimd.dma_start(out=oT[hp][64:128, Js],
                                                    in_=tmp[0:64, :])
                    if prev:
                        outproj(prev.pop())
                    prev.append(J)
                outproj(prev.pop())

    nc.compile()
    return nc


def get_module(mm="float16"):
    if mm not in _CACHE:
        _CACHE[mm] = _build_module(mm)
    return _CACHE[mm]


def _head_perm():
    """Within-head dim permutation: 16-pair blocks [x1 x2 x1 x2]."""
    p = []
    for blk in range(2):
        base = blk * 32
        p += [2 * (base // 2 + i) for i in range(16)]       # x1 of pairs
        p += [2 * (base // 2 + i) + 1 for i in range(16)]   # x2 of pairs
    return np.array(p)


def prep_core_inputs(inputs, mm="float16"):
    import ml_dtypes
    mdt = {"float16": np.float16, "bfloat16": ml_dtypes.bfloat16}.get(
        mm, np.float32)
    x = np.asarray(inputs["x"], dtype=np.float32)
    tp = np.asarray(inputs["token_positions"])
    wq = np.asarray(inputs["wq"], dtype=np.float32)
    wk = np.asarray(inputs["wk"], dtype=np.float32)
    wv = np.asarray(inputs["wv"], dtype=np.float32)
    wo = np.asarray(inputs["wo"], dtype=np.float32)

    perm = _head_perm()
    qi = np.arange(128)[None, :]
    ki = np.arange(128)[:, None]
    mask_add = np.where(qi < ki, np.float32(MASK_NEG),
                        np.float32(0.0)).astype(mdt)
    ident = np.eye(128, dtype=mdt)

    freqs = 1.0 / THETA ** (np.arange(0, D_K, 2, dtype=np.float32) / D_K)

    in_maps = []
    for c in range(N_CORES):
        b, g = divmod(c, 2)
        rows = slice(g * 512, (g + 1) * 512)
        wq_g = wq[rows].reshape(H_CORE, D_K, D_MODEL)[:, perm, :]
        wk_g = wk[rows].reshape(H_CORE, D_K, D_MODEL)[:, perm, :]

        pos = tp[b].astype(np.float32)
        ang = freqs[:, None] * pos[None, :]          # [32, S]
        cos32, sin32 = np.cos(ang), np.sin(ang)
        # permuted row l: l%32 < 16 -> x1 of pair (l%32 + 16*(l//32)),
        #                 else x2 of the same pair; x1 rows get -sin.
        cos64 = np.concatenate([cos32[0:16], cos32[0:16],
                                cos32[16:32], cos32[16:32]], axis=0)
        sin64 = np.concatenate([-sin32[0:16], sin32[0:16],
                                -sin32[16:32], sin32[16:32]], axis=0)
        cosT = np.tile(cos64, (2, 1))
        sinT = np.tile(sin64, (2, 1))

        in_maps.append({
            "xT": np.ascontiguousarray(x[b].T).astype(mdt),
            "wqT": np.ascontiguousarray(
                wq_g.reshape(512, D_MODEL).T).astype(mdt),
            "wkT": np.ascontiguousarray(
                wk_g.reshape(512, D_MODEL).T).astype(mdt),
            "wvT": np.ascontiguousarray(wv[rows].T).astype(mdt),
            "woT": np.ascontiguousarray(wo[:, rows].T).astype(np.float32),
            "cosT": np.ascontiguousarray(cosT),
            "sinT": np.ascontiguousarray(sinT),
            "maskA": mask_add,
            "identT": ident,
        })
    return in_maps


DEFAULT_MM = "float16"


def kernel(**inputs):
    from concourse.bass_utils import run_bass_kernel_spmd

    import os
    mm = os.environ.get("KMM", DEFAULT_MM)
    nc = get_module(mm)
    in_maps = prep_core_inputs(inputs, mm)
    res = run_bass_kernel_spmd(nc, in_maps, core_ids=list(range(N_CORES)))
    out = np.empty((B_FULL, S, D_MODEL), np.float32)
    for b in range(B_FULL):
        out[b] = res.results[2 * b]["out"] + res.results[2 * b + 1]["out"]
    return out
